# revision 6
# baseline (speedup 1.0000x reference)
"""GNN message-passing encoder (GatedGraphConv-style) on 8 Trainium2 NeuronCores.

Strategy (dst-sharded, gather-only, scatter-free):
  - Nodes are partitioned across 8 cores (12500 rows each). Each core owns the
    edges whose dst falls in its range.
  - Per layer: every core computes m = h @ W for its node slice. The m table is
    distributed via 4 per-quarter AllGathers (quarter q of every core's slice
    concatenates into "chunk" q of <=25000 rows, indexable by int16), so the
    gather over chunk q can start as soon as collective q lands and overlaps
    with the later collectives.
  - Messages are fetched with gpsimd.dma_gather. The per-core edge stream is
    laid out host-side as (chunk-major, window-minor) blocks padded to a
    uniform tile count B so the SPMD program is identical on all cores.
  - The scatter-add (segment sum over dst) is replaced by TensorE matmuls with
    data-built one-hot selection matrices: for each 128-slot tile an is_equal
    against an iota row yields onehot[slot, target]; psum accumulates the
    64x128 (feature x target) block over the B tiles of one (chunk, window)
    block; DVE adds flush psum into the aggregate. Pad slots carry a sentinel
    column value so they match nothing - no correction terms needed.
  - GRU cell runs on-chip per 128-row tile (PE matmuls + ACT sigmoid/tanh +
    DVE elementwise). Graph pooling is the same onehot-matmul trick over the
    sorted batch vector; each core emits a [256, 64] partial pooled sum and
    the host adds the 8 partials.
"""

import sys

for _p in ("/opt/trn_rl_repo", "/root/.axon_site/_ro/trn_rl_repo"):
    if _p not in sys.path:
        sys.path.insert(0, _p)

import numpy as np
import ml_dtypes

P = 128
N_CORES = 8
N_CHUNKS = 4
PAD_SENTINEL = 999.0

_cache = {}


def _wrap16(idx, channels=128):
    n = len(idx)
    a = np.asarray(idx, np.int16).reshape(n // 16, 16).T
    return np.ascontiguousarray(np.tile(a, (channels // 16, 1)))


def _host_prep(x, edge_index, batch):
    N, D = x.shape
    NPC = N // N_CORES
    QS = (NPC + N_CHUNKS - 1) // N_CHUNKS          # quarter size (last may be short)
    qsizes = [min(QS, NPC - q * QS) for q in range(N_CHUNKS)]
    NW = (NPC + P - 1) // P

    src = np.asarray(edge_index[0], np.int64)
    dst = np.asarray(edge_index[1], np.int64)
    batch = np.asarray(batch, np.int64)

    # quarter-relabeled table position: chunk q holds every core's q-th quarter
    src_core = src // NPC
    src_l = src % NPC
    src_chunk = np.minimum(src_l // QS, N_CHUNKS - 1)
    src_local = src_core * np.array(qsizes)[src_chunk] + (src_l - src_chunk * QS)

    dst_core = dst // NPC
    per_core = []
    B = 1
    for k in range(N_CORES):
        sel = dst_core == k
        s_loc = src_local[sel]
        c = src_chunk[sel]
        dl = dst[sel] - k * NPC
        w = dl // P
        key = c * NW + w
        order = np.argsort(key, kind="stable")
        s_loc, c, dl, w, key = s_loc[order], c[order], dl[order], w[order], key[order]
        cnt = np.bincount(key, minlength=N_CHUNKS * NW)
        B = max(B, int(-(-cnt.max() // P)))
        per_core.append((s_loc, dl, w, key, cnt))

    S = N_CHUNKS * NW * B * P  # padded slots per core
    cores = []
    for k in range(N_CORES):
        s_loc, dl, w, key, cnt = per_core[k]
        starts = np.zeros(N_CHUNKS * NW, np.int64)
        starts[1:] = np.cumsum(cnt)[:-1]
        rank = np.arange(len(s_loc)) - starts[key]
        pos = key * (B * P) + rank
        gidx = np.zeros(S, np.int16)
        gidx[pos] = s_loc.astype(np.int16)
        dcol = np.full(S, PAD_SENTINEL, np.float32)
        dcol[pos] = (dl - w * P).astype(np.float32)
        bslice = batch[k * NPC:(k + 1) * NPC]
        bc0 = np.full(NW * P, PAD_SENTINEL, np.float32)
        bc1 = np.full(NW * P, PAD_SENTINEL, np.float32)
        bc0[:NPC] = np.where(bslice < 128, bslice, PAD_SENTINEL)
        bc1[:NPC] = np.where(bslice >= 128, bslice - 128, PAD_SENTINEL)
        cores.append(dict(
            gidx=_wrap16(gidx),
            dcol=np.ascontiguousarray(
                dcol.reshape(S // P, P).T.astype(ml_dtypes.bfloat16)),
            bc0=np.ascontiguousarray(bc0.reshape(NW, P).T),
            bc1=np.ascontiguousarray(bc1.reshape(NW, P).T),
        ))
    return dict(N=N, D=D, NPC=NPC, QS=QS, qsizes=qsizes, NW=NW, B=B, S=S,
                cores=cores)


def _build_program(meta, n_layers):
    import concourse.bacc as bacc
    import concourse.mybir as mybir
    import concourse.tile as tile
    from concourse.library_config import mlp as mlp_lib

    N, D, NPC, QS, NW, B, S = (meta[z] for z in
                               ("N", "D", "NPC", "QS", "NW", "B", "S"))
    qsizes = meta["qsizes"]
    NT_LAST = NPC - (NW - 1) * P
    SEG_T = NW * B                       # tiles per chunk segment
    SEG_S = SEG_T * P                    # slots per chunk segment
    f32 = mybir.dt.float32
    bf16 = mybir.dt.bfloat16

    nc = bacc.Bacc("TRN2", target_bir_lowering=False, debug=False,
                   num_swdge_queues=4)

    xs = nc.dram_tensor("xs", [NPC, D], f32, kind="ExternalInput")
    gidx = nc.dram_tensor("gidx", [128, S // 16], mybir.dt.int16, kind="ExternalInput")
    dcol = nc.dram_tensor("dcol", [128, S // P], bf16, kind="ExternalInput")
    bc0 = nc.dram_tensor("bc0", [128, NW], f32, kind="ExternalInput")
    bc1 = nc.dram_tensor("bc1", [128, NW], f32, kind="ExternalInput")
    iota4 = nc.dram_tensor("iota4", [128, 4 * P], bf16, kind="ExternalInput")
    iotaf = nc.dram_tensor("iotaf", [128, P], f32, kind="ExternalInput")
    ident = nc.dram_tensor("ident", [128, P], f32, kind="ExternalInput")
    ones1 = nc.dram_tensor("ones1", [1, P], f32, kind="ExternalInput")
    wmat = nc.dram_tensor("wmat", [D, n_layers * D], f32, kind="ExternalInput")
    wihT = nc.dram_tensor("wihT", [D, 3 * D], f32, kind="ExternalInput")
    whhT = nc.dram_tensor("whhT", [D, 3 * D], f32, kind="ExternalInput")
    biasA = nc.dram_tensor("biasA", [1, 3 * D], f32, kind="ExternalInput")
    biasB = nc.dram_tensor("biasB", [1, D], f32, kind="ExternalInput")
    pooled = nc.dram_tensor("pooled", [256, D], f32, kind="ExternalOutput")

    with tile.TileContext(nc) as tc:
        with (
            tc.tile_pool(name="const", bufs=1) as cpool,
            tc.tile_pool(name="state", bufs=1) as spool,
            tc.tile_pool(name="gbuf", bufs=10) as gpool,
            tc.tile_pool(name="idx", bufs=2) as ipool,
            tc.tile_pool(name="oh", bufs=6) as opool,
            tc.tile_pool(name="tmp", bufs=8) as tpool,
            tc.tile_pool(name="ps", bufs=7, space="PSUM") as ppool,
            tc.tile_pool(name="dram", bufs=1, space="DRAM") as dpool,
        ):
            dcol_sb = spool.tile([128, S // P], bf16, tag="dcol")
            h_row = spool.tile([128, NW, D], f32, tag="hrow")
            hT = spool.tile([64, NW * P], f32, tag="hT")
            aggT = spool.tile([64, NW * P], f32, tag="aggT")
            iota4_sb = cpool.tile([128, 4 * P], bf16, tag="iota4")
            iotaf_sb = cpool.tile([128, P], f32, tag="iotaf")
            ident_sb = cpool.tile([128, P], f32, tag="ident")
            ones1_sb = cpool.tile([1, P], f32, tag="ones1")
            wmat_sb = cpool.tile([D, n_layers * D], f32, tag="wmat")
            wihT_sb = cpool.tile([D, 3 * D], f32, tag="wihT")
            whhT_sb = cpool.tile([D, 3 * D], f32, tag="whhT")
            biasA_sb = cpool.tile([1, 3 * D], f32, tag="biasA")
            biasB_sb = cpool.tile([1, D], f32, tag="biasB")
            bc0_sb = cpool.tile([128, NW], f32, tag="bc0")
            bc1_sb = cpool.tile([128, NW], f32, tag="bc1")

            nc.sync.dma_start(dcol_sb[:], dcol[:])
            nc.sync.dma_start(iota4_sb[:], iota4[:])
            nc.sync.dma_start(iotaf_sb[:], iotaf[:])
            nc.sync.dma_start(ident_sb[:], ident[:])
            nc.sync.dma_start(ones1_sb[:], ones1[:])
            nc.sync.dma_start(wmat_sb[:], wmat[:])
            nc.sync.dma_start(wihT_sb[:], wihT[:])
            nc.sync.dma_start(whhT_sb[:], whhT[:])
            nc.sync.dma_start(biasA_sb[:], biasA[:])
            nc.sync.dma_start(biasB_sb[:], biasB[:])
            nc.sync.dma_start(bc0_sb[:], bc0[:])
            nc.sync.dma_start(bc1_sb[:], bc1[:])

            # ---- init: load x -> h_row, build hT ----
            nc.gpsimd.memset(h_row[:], 0.0)
            nfull = NPC // P
            nc.sync.dma_start(
                h_row[:, :nfull, :],
                xs[:nfull * P, :].rearrange("(t p) d -> p t d", p=P),
            )
            if NT_LAST < P:
                nc.sync.dma_start(h_row[:NT_LAST, nfull, :], xs[nfull * P:, :])
            for t in range(NW):
                pst = ppool.tile([64, P], f32, tag="ps")
                nc.tensor.transpose(pst[:], h_row[:, t, :], ident_sb[:])
                nc.scalar.activation(hT[:, t * P:(t + 1) * P], pst[:],
                                     mybir.ActivationFunctionType.Copy)

            nc.gpsimd.load_library(mlp_lib)

            qstart = [sum(qsizes[:q]) for q in range(N_CHUNKS)]
            for layer in range(n_layers):
                m_bounce = dpool.tile([NPC, D], f32, tag=f"mb{layer}")
                m_chunks = []
                for q in range(N_CHUNKS):
                    m_chunk_q = dpool.tile([N_CORES * qsizes[q], D], f32,
                                           addr_space="Shared",
                                           tag=f"mf{layer}_{q}")
                    m_chunks.append(m_chunk_q)
                # ---- phase M ----
                for t in range(NW):
                    psm = ppool.tile([128, D], f32, tag="ps")
                    nc.tensor.matmul(psm[:], lhsT=hT[:, t * P:(t + 1) * P],
                                     rhs=wmat_sb[:, layer * D:(layer + 1) * D],
                                     start=True, stop=True)
                    mt = tpool.tile([128, D], f32, tag="mt")
                    nc.scalar.activation(mt[:], psm[:],
                                         mybir.ActivationFunctionType.Copy)
                    rows = P if t < NW - 1 else NT_LAST
                    nc.sync.dma_start(m_bounce[t * P:t * P + rows, :], mt[:rows, :])
                for q in range(N_CHUNKS):
                    nc.gpsimd.collective_compute(
                        "AllGather", mybir.AluOpType.bypass,
                        ins=[m_bounce[qstart[q]:qstart[q] + qsizes[q], :]],
                        outs=[m_chunks[q][:]],
                        replica_groups=[list(range(N_CORES))],
                    )

                nc.gpsimd.memset(aggT[:], 0.0)

                # ---- phase G ----
                for c in range(N_CHUNKS):
                    idxb = ipool.tile([128, SEG_S // 16], mybir.dt.int16, tag="idxb")
                    nc.sync.dma_start(
                        idxb[:], gidx[:, c * (SEG_S // 16):(c + 1) * (SEG_S // 16)])
                    t0 = c * SEG_T
                    n_inst = (SEG_T + 7) // 8
                    for j in range(n_inst):
                        tiles_this = min(8, SEG_T - j * 8)
                        ni = tiles_this * P
                        gb = gpool.tile([128, 8, D], f32, tag="gb")
                        off = j * 64  # int16 cols within chunk: 8 tiles * 128/16
                        nc.gpsimd.dma_gather(
                            gb[:, :tiles_this, :], m_chunks[c][:],
                            idxb[:, off:off + ni // 16],
                            ni, ni, D, queue_num=j % 4,
                        )
                        for g in range(0, tiles_this, 4):
                            gsz = min(4, tiles_this - g)
                            oht = opool.tile([128, 4 * P], f32, tag="oh")
                            gt = t0 + j * 8 + g
                            nc.vector.tensor_tensor(
                                out=oht[:].rearrange("p (a b) -> p a b", a=4)[:, :gsz, :],
                                in0=dcol_sb[:, gt:gt + gsz].to_broadcast([128, gsz, P]),
                                in1=iota4_sb[:].rearrange("p (a b) -> p a b", a=4)[:, :gsz, :],
                                op=mybir.AluOpType.is_equal,
                            )
                            for u in range(gsz):
                                tt = j * 8 + g + u
                                w = tt // B
                                tib = tt % B
                                if tib == 0:
                                    _live_psr[0] = ppool.tile(
                                        [64, P], f32, tag="ps", name=f"psr{c}_{tt}")
                                psr = _live_psr[0]
                                nc.tensor.matmul(
                                    psr[:], lhsT=gb[:, g + u, :],
                                    rhs=oht[:, u * P:(u + 1) * P],
                                    start=(tib == 0), stop=(tib == B - 1),
                                )
                                if tib == B - 1:
                                    nc.vector.tensor_tensor(
                                        out=aggT[:, w * P:(w + 1) * P],
                                        in0=aggT[:, w * P:(w + 1) * P],
                                        in1=psr[:], op=mybir.AluOpType.add,
                                    )

                # ---- phase U: GRU ----
                for t in range(NW):
                    sl = slice(t * P, (t + 1) * P)
                    psA = ppool.tile([128, 3 * D], f32, tag="ps")
                    nc.tensor.matmul(psA[:], lhsT=aggT[:, sl], rhs=wihT_sb[:],
                                     start=True, stop=False)
                    nc.tensor.matmul(psA[:, 0:2 * D], lhsT=hT[:, sl],
                                     rhs=whhT_sb[:, 0:2 * D],
                                     start=False, stop=False)
                    nc.tensor.matmul(psA[:], lhsT=ones1_sb[:], rhs=biasA_sb[:],
                                     start=False, stop=True)
                    psB = ppool.tile([128, D], f32, tag="ps")
                    nc.tensor.matmul(psB[:], lhsT=hT[:, sl],
                                     rhs=whhT_sb[:, 2 * D:3 * D],
                                     start=True, stop=False)
                    nc.tensor.matmul(psB[:], lhsT=ones1_sb[:], rhs=biasB_sb[:],
                                     start=False, stop=True)
                    r = tpool.tile([128, D], f32, tag="r")
                    z = tpool.tile([128, D], f32, tag="z")
                    nn = tpool.tile([128, D], f32, tag="nn")
                    t1 = tpool.tile([128, D], f32, tag="t1")
                    nc.scalar.activation(r[:], psA[:, 0:D],
                                         mybir.ActivationFunctionType.Sigmoid)
                    nc.scalar.activation(z[:], psA[:, D:2 * D],
                                         mybir.ActivationFunctionType.Sigmoid)
                    nc.vector.tensor_tensor(out=t1[:], in0=r[:], in1=psB[:],
                                            op=mybir.AluOpType.mult)
                    nc.vector.tensor_tensor(out=t1[:], in0=t1[:],
                                            in1=psA[:, 2 * D:3 * D],
                                            op=mybir.AluOpType.add)
                    nc.scalar.activation(nn[:], t1[:],
                                         mybir.ActivationFunctionType.Tanh)
                    nc.vector.tensor_tensor(out=t1[:], in0=h_row[:, t, :],
                                            in1=nn[:], op=mybir.AluOpType.subtract)
                    nc.vector.tensor_tensor(out=t1[:], in0=z[:], in1=t1[:],
                                            op=mybir.AluOpType.mult)
                    nc.vector.tensor_tensor(out=h_row[:, t, :], in0=nn[:],
                                            in1=t1[:], op=mybir.AluOpType.add)
                    if layer < n_layers - 1:
                        pst = ppool.tile([64, P], f32, tag="ps")
                        nc.tensor.transpose(pst[:], h_row[:, t, :], ident_sb[:])
                        nc.scalar.activation(hT[:, sl], pst[:],
                                             mybir.ActivationFunctionType.Copy)

            # ---- pooling ----
            psP0 = ppool.tile([128, D], f32, tag="ps")
            psP1 = ppool.tile([128, D], f32, tag="ps")
            for t in range(NW):
                oh0 = opool.tile([128, 4 * P], f32, tag="oh")
                nc.vector.tensor_tensor(
                    out=oh0[:, 0:P],
                    in0=bc0_sb[:, t:t + 1].to_broadcast([128, P]),
                    in1=iotaf_sb[:], op=mybir.AluOpType.is_equal)
                nc.vector.tensor_tensor(
                    out=oh0[:, P:2 * P],
                    in0=bc1_sb[:, t:t + 1].to_broadcast([128, P]),
                    in1=iotaf_sb[:], op=mybir.AluOpType.is_equal)
                nc.tensor.matmul(psP0[:], lhsT=oh0[:, 0:P], rhs=h_row[:, t, :],
                                 start=(t == 0), stop=(t == NW - 1))
                nc.tensor.matmul(psP1[:], lhsT=oh0[:, P:2 * P], rhs=h_row[:, t, :],
                                 start=(t == 0), stop=(t == NW - 1))
            po = tpool.tile([128, D], f32, tag="po")
            nc.scalar.activation(po[:], psP0[:], mybir.ActivationFunctionType.Copy)
            nc.sync.dma_start(pooled[0:128, :], po[:])
            po2 = tpool.tile([128, D], f32, tag="po")
            nc.scalar.activation(po2[:], psP1[:], mybir.ActivationFunctionType.Copy)
            nc.sync.dma_start(pooled[128:256, :], po2[:])

    nc.compile()
    return nc


_live_psr = [None]


def kernel(x, edge_index, batch, weight, W_ih, W_hh, b_ih, b_hh,
           _trace=False):
    from concourse.bass_utils import run_bass_kernel_spmd

    x = np.asarray(x, np.float32)
    weight = np.asarray(weight, np.float32)
    W_ih = np.asarray(W_ih, np.float32)
    W_hh = np.asarray(W_hh, np.float32)
    b_ih = np.asarray(b_ih, np.float32)
    b_hh = np.asarray(b_hh, np.float32)
    N, D = x.shape
    n_layers = weight.shape[0]
    NPC = N // N_CORES

    meta = _host_prep(x, edge_index, batch)
    key = (N, D, n_layers, meta["B"])
    if key not in _cache:
        _cache[key] = _build_program(meta, n_layers)
    nc = _cache[key]

    iota_b = np.tile(np.arange(P, dtype=np.float32),
                     (128, 4)).astype(ml_dtypes.bfloat16)
    iota_f = np.tile(np.arange(P, dtype=np.float32), (128, 1))
    ident_np = np.eye(P, dtype=np.float32)
    ones1_np = np.ones((1, P), np.float32)
    wmat_np = np.concatenate([weight[i] for i in range(n_layers)], axis=1)
    wihT_np = np.ascontiguousarray(W_ih.T)
    whhT_np = np.ascontiguousarray(W_hh.T)
    biasA_np = np.concatenate([
        b_ih[0:D] + b_hh[0:D], b_ih[D:2 * D] + b_hh[D:2 * D],
        b_ih[2 * D:3 * D]]).reshape(1, 3 * D).astype(np.float32)
    biasB_np = b_hh[2 * D:3 * D].reshape(1, D).astype(np.float32)

    in_maps = []
    for k in range(N_CORES):
        ck = meta["cores"][k]
        in_maps.append(dict(
            xs=np.ascontiguousarray(x[k * NPC:(k + 1) * NPC]),
            gidx=ck["gidx"], dcol=ck["dcol"], bc0=ck["bc0"], bc1=ck["bc1"],
            iota4=iota_b, iotaf=iota_f, ident=ident_np, ones1=ones1_np,
            wmat=wmat_np, wihT=wihT_np, whhT=whhT_np, biasA=biasA_np,
            biasB=biasB_np,
        ))

    res = run_bass_kernel_spmd(nc, in_maps, core_ids=list(range(N_CORES)),
                               trace=_trace)
    out = np.zeros((256, D), np.float32)
    for k in range(N_CORES):
        out += res.results[k]["pooled"]
    kernel._last_exec_time_ns = res.exec_time_ns
    return out


# revision 7
# speedup vs baseline: 2.0963x; 2.0963x over previous
"""GNN message-passing encoder (GatedGraphConv-style) on 8 Trainium2 NeuronCores.

Strategy (dst-sharded, gather-only, scatter-free):
  - Nodes are partitioned across 8 cores (12500 rows each); each core owns the
    edges whose dst falls in its range.
  - Per layer every core computes m = h @ W for its slice; the table is
    distributed via 4 per-quarter AllGathers (chunk q = quarter q of every
    core, <=25000 rows so gpsimd.dma_gather's int16 indices reach it). The
    collectives fire while the previous layer's GRU wave is still running.
  - The per-core edge stream is laid out host-side as window-group-major
    blocks: groups of WG=4 dst-windows (128 nodes each), within a group the 4
    src-chunks back to back, every (chunk, window) block padded to a uniform B
    tiles of 128 slots, so the SPMD program is identical on all cores.
  - The scatter-add (segment-sum over dst) is replaced by TensorE matmuls with
    data-built one-hot selection matrices (is_equal of a per-slot dst-column
    stream against an iota row). All 4*B tiles of one window accumulate into
    one PSUM bank; ScalarE copies the finished window into SBUF. Pad slots
    carry a sentinel column so they match nothing.
  - The GRU cell (PE matmuls + ACT sigmoid/tanh + DVE elementwise) for a
    window group runs while the next group is being gathered; the next layer's
    m tile is produced right after each GRU tile. Graph pooling uses the same
    onehot-matmul trick over the sorted batch vector; each core emits a
    [256, 64] partial pooled sum and the host adds the 8 partials.
"""

import sys

for _p in ("/opt/trn_rl_repo", "/root/.axon_site/_ro/trn_rl_repo"):
    if _p not in sys.path:
        sys.path.insert(0, _p)

import numpy as np
import ml_dtypes

P = 128
N_CORES = 8
N_CHUNKS = 4
WG = 4                 # windows per group
PAD_SENTINEL = 999.0

_cache = {}


def _wrap16(idx, channels=128):
    n = len(idx)
    a = np.asarray(idx, np.int16).reshape(n // 16, 16).T
    return np.ascontiguousarray(np.tile(a, (channels // 16, 1)))


def _host_prep(x, edge_index, batch):
    N, D = x.shape
    NPC = N // N_CORES
    QS = (NPC + N_CHUNKS - 1) // N_CHUNKS
    qsizes = [min(QS, NPC - q * QS) for q in range(N_CHUNKS)]
    NW = (NPC + P - 1) // P
    NWG = (NW + WG - 1) // WG

    src = np.asarray(edge_index[0], np.int64)
    dst = np.asarray(edge_index[1], np.int64)
    batch = np.asarray(batch, np.int64)

    # quarter-relabeled table position: chunk q holds every core's q-th quarter
    src_core = src // NPC
    src_l = src % NPC
    src_chunk = np.minimum(src_l // QS, N_CHUNKS - 1)
    src_local = src_core * np.array(qsizes)[src_chunk] + (src_l - src_chunk * QS)

    # stream block order: (window_group, chunk, window_in_group)
    def block_id(c, w):
        return (w // WG) * (N_CHUNKS * WG) + c * WG + (w % WG)

    dst_core = dst // NPC
    per_core = []
    B = 1
    n_blocks = NWG * N_CHUNKS * WG  # includes ghost blocks of a short last group
    for k in range(N_CORES):
        sel = dst_core == k
        s_loc = src_local[sel]
        c = src_chunk[sel]
        dl = dst[sel] - k * NPC
        w = dl // P
        key = block_id(c, w)
        order = np.argsort(key, kind="stable")
        s_loc, dl, w, key = s_loc[order], dl[order], w[order], key[order]
        cnt = np.bincount(key, minlength=n_blocks)
        B = max(B, int(-(-cnt.max() // P)))
        per_core.append((s_loc, dl, w, key, cnt))

    S = n_blocks * B * P  # padded slots per core (ghost blocks included)
    cores = []
    for k in range(N_CORES):
        s_loc, dl, w, key, cnt = per_core[k]
        starts = np.zeros(n_blocks, np.int64)
        starts[1:] = np.cumsum(cnt)[:-1]
        rank = np.arange(len(s_loc)) - starts[key]
        pos = key * (B * P) + rank
        gidx = np.zeros(S, np.int16)
        gidx[pos] = s_loc.astype(np.int16)
        dcol = np.full(S, PAD_SENTINEL, np.float32)
        dcol[pos] = (dl - w * P).astype(np.float32)
        bslice = batch[k * NPC:(k + 1) * NPC]
        bc0 = np.full(NW * P, PAD_SENTINEL, np.float32)
        bc1 = np.full(NW * P, PAD_SENTINEL, np.float32)
        bc0[:NPC] = np.where(bslice < 128, bslice, PAD_SENTINEL)
        bc1[:NPC] = np.where(bslice >= 128, bslice - 128, PAD_SENTINEL)
        cores.append(dict(
            gidx=_wrap16(gidx),
            dcol=np.ascontiguousarray(
                dcol.reshape(S // P, P).T.astype(ml_dtypes.bfloat16)),
            bc0=np.ascontiguousarray(bc0.reshape(NW, P).T),
            bc1=np.ascontiguousarray(bc1.reshape(NW, P).T),
        ))
    return dict(N=N, D=D, NPC=NPC, QS=QS, qsizes=qsizes, NW=NW, NWG=NWG, B=B,
                S=S, cores=cores)


def _build_program(meta, n_layers):
    import concourse.bacc as bacc
    import concourse.mybir as mybir
    import concourse.tile as tile
    from concourse.library_config import mlp as mlp_lib

    N, D, NPC, QS, NW, NWG, B, S = (meta[z] for z in
                                    ("N", "D", "NPC", "QS", "NW", "NWG", "B", "S"))
    qsizes = meta["qsizes"]
    NT_LAST = NPC - (NW - 1) * P
    f32 = mybir.dt.float32
    bf16 = mybir.dt.bfloat16
    AF = mybir.ActivationFunctionType

    nc = bacc.Bacc("TRN2", target_bir_lowering=False, debug=False,
                   num_swdge_queues=4)

    xs = nc.dram_tensor("xs", [NPC, D], f32, kind="ExternalInput")
    gidx = nc.dram_tensor("gidx", [128, S // 16], mybir.dt.int16, kind="ExternalInput")
    dcol = nc.dram_tensor("dcol", [128, S // P], bf16, kind="ExternalInput")
    bc0 = nc.dram_tensor("bc0", [128, NW], f32, kind="ExternalInput")
    bc1 = nc.dram_tensor("bc1", [128, NW], f32, kind="ExternalInput")
    iota4 = nc.dram_tensor("iota4", [128, 4 * P], bf16, kind="ExternalInput")
    iotaf = nc.dram_tensor("iotaf", [128, P], f32, kind="ExternalInput")
    ident = nc.dram_tensor("ident", [128, P], f32, kind="ExternalInput")
    ones1 = nc.dram_tensor("ones1", [1, P], f32, kind="ExternalInput")
    wmat = nc.dram_tensor("wmat", [D, n_layers * D], f32, kind="ExternalInput")
    wihT = nc.dram_tensor("wihT", [D, 3 * D], f32, kind="ExternalInput")
    whhT = nc.dram_tensor("whhT", [D, 3 * D], f32, kind="ExternalInput")
    biasA = nc.dram_tensor("biasA", [1, 3 * D], f32, kind="ExternalInput")
    biasB = nc.dram_tensor("biasB", [1, D], f32, kind="ExternalInput")
    pooled = nc.dram_tensor("pooled", [256, D], f32, kind="ExternalOutput")

    with tile.TileContext(nc) as tc:
        with (
            tc.tile_pool(name="const", bufs=1) as cpool,
            tc.tile_pool(name="state", bufs=1) as spool,
            tc.tile_pool(name="gbuf", bufs=14) as gpool,
            tc.tile_pool(name="oh", bufs=8) as opool,
            tc.tile_pool(name="aggw", bufs=8) as apool,
            tc.tile_pool(name="tmp", bufs=8) as tpool,
            tc.tile_pool(name="red", bufs=4, space="PSUM") as rpool,
            tc.tile_pool(name="gru", bufs=4, space="PSUM") as upool,
            tc.tile_pool(name="dram", bufs=1, space="DRAM") as dpool,
        ):
            gidx_sb = spool.tile([128, S // 16], mybir.dt.int16, tag="gidx")
            dcol_sb = spool.tile([128, S // P], bf16, tag="dcol")
            h_row = spool.tile([128, NW, D], f32, tag="hrow")
            hT = spool.tile([64, NW * P], f32, tag="hT")
            iota4_sb = cpool.tile([128, 4 * P], bf16, tag="iota4")
            iotaf_sb = cpool.tile([128, P], f32, tag="iotaf")
            ident_sb = cpool.tile([128, P], f32, tag="ident")
            ones1_sb = cpool.tile([1, P], f32, tag="ones1")
            wmat_sb = cpool.tile([D, n_layers * D], f32, tag="wmat")
            wihT_sb = cpool.tile([D, 3 * D], f32, tag="wihT")
            whhT_sb = cpool.tile([D, 3 * D], f32, tag="whhT")
            biasA_sb = cpool.tile([1, 3 * D], f32, tag="biasA")
            biasB_sb = cpool.tile([1, D], f32, tag="biasB")
            bc0_sb = cpool.tile([128, NW], f32, tag="bc0")
            bc1_sb = cpool.tile([128, NW], f32, tag="bc1")

            nc.sync.dma_start(gidx_sb[:], gidx[:])
            nc.sync.dma_start(dcol_sb[:], dcol[:])
            nc.sync.dma_start(iota4_sb[:], iota4[:])
            nc.sync.dma_start(iotaf_sb[:], iotaf[:])
            nc.sync.dma_start(ident_sb[:], ident[:])
            nc.sync.dma_start(ones1_sb[:], ones1[:])
            nc.sync.dma_start(wmat_sb[:], wmat[:])
            nc.sync.dma_start(wihT_sb[:], wihT[:])
            nc.sync.dma_start(whhT_sb[:], whhT[:])
            nc.sync.dma_start(biasA_sb[:], biasA[:])
            nc.sync.dma_start(biasB_sb[:], biasB[:])
            nc.sync.dma_start(bc0_sb[:], bc0[:])
            nc.sync.dma_start(bc1_sb[:], bc1[:])

            qstart = [sum(qsizes[:q]) for q in range(N_CHUNKS)]
            m_bounces = []
            m_chunks = []
            for layer in range(n_layers):
                mb = dpool.tile([NPC, D], f32, tag=f"mb{layer}", name=f"mb{layer}")
                chs = []
                for q in range(N_CHUNKS):
                    ch = dpool.tile([N_CORES * qsizes[q], D], f32,
                                    addr_space="Shared", tag=f"mf{layer}_{q}",
                                    name=f"mf{layer}_{q}")
                    chs.append(ch)
                m_bounces.append(mb)
                m_chunks.append(chs)

            def emit_m_tile(layer, t, rows):
                psm = upool.tile([128, D], f32, tag="gru", name=f"psm{layer}_{t}")
                nc.tensor.matmul(psm[:], lhsT=hT[:, t * P:(t + 1) * P],
                                 rhs=wmat_sb[:, layer * D:(layer + 1) * D],
                                 start=True, stop=True)
                mt = tpool.tile([128, D], f32, tag="mt", name=f"mt{layer}_{t}")
                nc.scalar.activation(mt[:], psm[:], AF.Copy)
                nc.sync.dma_start(
                    m_bounces[layer][t * P:t * P + rows, :], mt[:rows, :])

            def emit_collectives(layer):
                for q in range(N_CHUNKS):
                    nc.gpsimd.collective_compute(
                        "AllGather", mybir.AluOpType.bypass,
                        ins=[m_bounces[layer][qstart[q]:qstart[q] + qsizes[q], :]],
                        outs=[m_chunks[layer][q][:]],
                        replica_groups=[list(range(N_CORES))],
                    )

            # ---- init: load x -> h_row, build hT, layer-0 m + collectives ----
            nc.gpsimd.memset(h_row[:], 0.0)
            nfull = NPC // P
            nc.sync.dma_start(
                h_row[:, :nfull, :],
                xs[:nfull * P, :].rearrange("(t p) d -> p t d", p=P),
            )
            if NT_LAST < P:
                nc.sync.dma_start(h_row[:NT_LAST, nfull, :], xs[nfull * P:, :])
            for t in range(NW):
                pst = upool.tile([64, P], f32, tag="gru", name=f"pst_i{t}")
                nc.tensor.transpose(pst[:], h_row[:, t, :], ident_sb[:])
                nc.scalar.activation(hT[:, t * P:(t + 1) * P], pst[:], AF.Copy)
                emit_m_tile(0, t, P if t < NW - 1 else NT_LAST)
            emit_collectives(0)

            nc.gpsimd.load_library(mlp_lib)

            inst_q = [0]

            def emit_group_gathers(layer, wg):
                """Gather + onehot + reduce matmuls for one window group."""
                wsz = min(WG, NW - wg * WG)
                aggws = {}
                psums = {}
                for c in range(N_CHUNKS):
                    g0 = (wg * (N_CHUNKS * WG) + c * WG) * B  # first tile of block group
                    gtiles = WG * B  # tiles incl ghost windows (gathered, unused)
                    rtiles = wsz * B
                    # gather instructions over the real tiles
                    j = 0
                    while j < rtiles:
                        tiles_this = min(8, rtiles - j)
                        ni = tiles_this * P
                        gb = gpool.tile([128, 8, D], f32, tag="gb",
                                        name=f"gb{layer}_{wg}_{c}_{j}")
                        off = (g0 + j) * 8
                        nc.gpsimd.dma_gather(
                            gb[:, :tiles_this, :], m_chunks[layer][c][:],
                            gidx_sb[:, off:off + ni // 16],
                            ni, ni, D, queue_num=inst_q[0] % 4,
                        )
                        inst_q[0] += 1
                        for g in range(0, tiles_this, 4):
                            gsz = min(4, tiles_this - g)
                            oht = opool.tile([128, 4 * P], f32, tag="oh",
                                             name=f"oh{layer}_{wg}_{c}_{j}_{g}")
                            gt = g0 + j + g
                            nc.vector.tensor_tensor(
                                out=oht[:].rearrange("p (a b) -> p a b", a=4)[:, :gsz, :],
                                in0=dcol_sb[:, gt:gt + gsz].to_broadcast([128, gsz, P]),
                                in1=iota4_sb[:].rearrange("p (a b) -> p a b", a=4)[:, :gsz, :],
                                op=mybir.AluOpType.is_equal,
                            )
                            for u in range(gsz):
                                tt = j + g + u        # tile within this (c, wg) group
                                wi = tt // B          # window within group
                                tib = tt % B
                                w = wg * WG + wi
                                if c == 0 and tib == 0:
                                    psums[wi] = rpool.tile(
                                        [64, P], f32, tag="red",
                                        name=f"psr{layer}_{wg}_{wi}")
                                nc.tensor.matmul(
                                    psums[wi][:], lhsT=gb[:, g + u, :],
                                    rhs=oht[:, u * P:(u + 1) * P],
                                    start=(c == 0 and tib == 0),
                                    stop=(c == N_CHUNKS - 1 and tib == B - 1),
                                )
                                if c == N_CHUNKS - 1 and tib == B - 1:
                                    aggw = apool.tile([64, P], f32, tag="aggw",
                                                      name=f"aggw{layer}_{w}")
                                    nc.scalar.activation(aggw[:], psums[wi][:],
                                                         AF.Copy)
                                    aggws[wi] = aggw
                        j += tiles_this
                return aggws

            def emit_gru_tile(layer, t, aggw):
                sl = slice(t * P, (t + 1) * P)
                psA = upool.tile([128, 3 * D], f32, tag="gru",
                                 name=f"psA{layer}_{t}")
                nc.tensor.matmul(psA[:], lhsT=aggw[:], rhs=wihT_sb[:],
                                 start=True, stop=False)
                nc.tensor.matmul(psA[:, 0:2 * D], lhsT=hT[:, sl],
                                 rhs=whhT_sb[:, 0:2 * D], start=False, stop=False)
                nc.tensor.matmul(psA[:], lhsT=ones1_sb[:], rhs=biasA_sb[:],
                                 start=False, stop=True)
                psB = upool.tile([128, D], f32, tag="gru", name=f"psB{layer}_{t}")
                nc.tensor.matmul(psB[:], lhsT=hT[:, sl],
                                 rhs=whhT_sb[:, 2 * D:3 * D], start=True, stop=False)
                nc.tensor.matmul(psB[:], lhsT=ones1_sb[:], rhs=biasB_sb[:],
                                 start=False, stop=True)
                r = tpool.tile([128, D], f32, tag="r", name=f"r{layer}_{t}")
                z = tpool.tile([128, D], f32, tag="z", name=f"z{layer}_{t}")
                nn = tpool.tile([128, D], f32, tag="nn", name=f"nn{layer}_{t}")
                t1 = tpool.tile([128, D], f32, tag="t1", name=f"t1{layer}_{t}")
                nc.scalar.activation(r[:], psA[:, 0:D], AF.Sigmoid)
                nc.scalar.activation(z[:], psA[:, D:2 * D], AF.Sigmoid)
                nc.vector.tensor_tensor(out=t1[:], in0=r[:], in1=psB[:],
                                        op=mybir.AluOpType.mult)
                nc.vector.tensor_tensor(out=t1[:], in0=t1[:],
                                        in1=psA[:, 2 * D:3 * D],
                                        op=mybir.AluOpType.add)
                nc.scalar.activation(nn[:], t1[:], AF.Tanh)
                nc.vector.tensor_tensor(out=t1[:], in0=h_row[:, t, :], in1=nn[:],
                                        op=mybir.AluOpType.subtract)
                nc.vector.tensor_tensor(out=t1[:], in0=z[:], in1=t1[:],
                                        op=mybir.AluOpType.mult)
                nc.vector.tensor_tensor(out=h_row[:, t, :], in0=nn[:], in1=t1[:],
                                        op=mybir.AluOpType.add)
                if layer < n_layers - 1:
                    pst = upool.tile([64, P], f32, tag="gru",
                                     name=f"pst{layer}_{t}")
                    nc.tensor.transpose(pst[:], h_row[:, t, :], ident_sb[:])
                    nc.scalar.activation(hT[:, sl], pst[:], AF.Copy)
                    emit_m_tile(layer + 1, t, P if t < NW - 1 else NT_LAST)

            for layer in range(n_layers):
                for wg in range(NWG):
                    aggws = emit_group_gathers(layer, wg)
                    wsz = min(WG, NW - wg * WG)
                    for wi in range(wsz):
                        emit_gru_tile(layer, wg * WG + wi, aggws[wi])
                if layer < n_layers - 1:
                    emit_collectives(layer + 1)

            # ---- pooling ----
            psP0 = upool.tile([128, D], f32, tag="gru", name="psP0")
            psP1 = upool.tile([128, D], f32, tag="gru", name="psP1")
            for t in range(NW):
                oh0 = opool.tile([128, 4 * P], f32, tag="oh", name=f"ohp{t}")
                nc.vector.tensor_tensor(
                    out=oh0[:, 0:P],
                    in0=bc0_sb[:, t:t + 1].to_broadcast([128, P]),
                    in1=iotaf_sb[:], op=mybir.AluOpType.is_equal)
                nc.vector.tensor_tensor(
                    out=oh0[:, P:2 * P],
                    in0=bc1_sb[:, t:t + 1].to_broadcast([128, P]),
                    in1=iotaf_sb[:], op=mybir.AluOpType.is_equal)
                nc.tensor.matmul(psP0[:], lhsT=oh0[:, 0:P], rhs=h_row[:, t, :],
                                 start=(t == 0), stop=(t == NW - 1))
                nc.tensor.matmul(psP1[:], lhsT=oh0[:, P:2 * P], rhs=h_row[:, t, :],
                                 start=(t == 0), stop=(t == NW - 1))
            po = tpool.tile([128, D], f32, tag="po", name="po")
            nc.scalar.activation(po[:], psP0[:], AF.Copy)
            nc.sync.dma_start(pooled[0:128, :], po[:])
            po2 = tpool.tile([128, D], f32, tag="po", name="po2")
            nc.scalar.activation(po2[:], psP1[:], AF.Copy)
            nc.sync.dma_start(pooled[128:256, :], po2[:])

    nc.compile()
    return nc


def kernel(x, edge_index, batch, weight, W_ih, W_hh, b_ih, b_hh,
           _trace=False):
    from concourse.bass_utils import run_bass_kernel_spmd

    x = np.asarray(x, np.float32)
    weight = np.asarray(weight, np.float32)
    W_ih = np.asarray(W_ih, np.float32)
    W_hh = np.asarray(W_hh, np.float32)
    b_ih = np.asarray(b_ih, np.float32)
    b_hh = np.asarray(b_hh, np.float32)
    N, D = x.shape
    n_layers = weight.shape[0]
    NPC = N // N_CORES

    meta = _host_prep(x, edge_index, batch)
    key = (N, D, n_layers, meta["B"])
    if key not in _cache:
        _cache[key] = _build_program(meta, n_layers)
    nc = _cache[key]

    iota_b = np.tile(np.arange(P, dtype=np.float32),
                     (128, 4)).astype(ml_dtypes.bfloat16)
    iota_f = np.tile(np.arange(P, dtype=np.float32), (128, 1))
    ident_np = np.eye(P, dtype=np.float32)
    ones1_np = np.ones((1, P), np.float32)
    wmat_np = np.concatenate([weight[i] for i in range(n_layers)], axis=1)
    wihT_np = np.ascontiguousarray(W_ih.T)
    whhT_np = np.ascontiguousarray(W_hh.T)
    biasA_np = np.concatenate([
        b_ih[0:D] + b_hh[0:D], b_ih[D:2 * D] + b_hh[D:2 * D],
        b_ih[2 * D:3 * D]]).reshape(1, 3 * D).astype(np.float32)
    biasB_np = b_hh[2 * D:3 * D].reshape(1, D).astype(np.float32)

    in_maps = []
    for k in range(N_CORES):
        ck = meta["cores"][k]
        in_maps.append(dict(
            xs=np.ascontiguousarray(x[k * NPC:(k + 1) * NPC]),
            gidx=ck["gidx"], dcol=ck["dcol"], bc0=ck["bc0"], bc1=ck["bc1"],
            iota4=iota_b, iotaf=iota_f, ident=ident_np, ones1=ones1_np,
            wmat=wmat_np, wihT=wihT_np, whhT=whhT_np, biasA=biasA_np,
            biasB=biasB_np,
        ))

    res = run_bass_kernel_spmd(nc, in_maps, core_ids=list(range(N_CORES)),
                               trace=_trace)
    out = np.zeros((256, D), np.float32)
    for k in range(N_CORES):
        out += res.results[k]["pooled"]
    kernel._last_exec_time_ns = res.exec_time_ns
    return out
